# revision 53
# baseline (speedup 1.0000x reference)
"""Trainium2 Bass kernel for nn_Attention_LR_65249143160949 (cross-attention block).

Sharding: 8 cores = 4 batches x 2 token-halves (1152 tokens each). Each core
computes k/v for its whole batch (cheap MQA single head, duplicated within the
pair) and q/attention/output for its own tokens. The host permutes tokens so
each core's own rows come first -> identical SPMD program, no collectives.

On-chip layout: features on partitions, tokens on the free axis (matches the
channels-first HBM layout; no input transpose). LayerNorm is folded into the
projections: q = rs_i * (x @ Wq' - mu_i * colsum(Wq')), with Wq' pre-scaled on
the host; k/v analogous (rank-1 -colsum*mu matmul accumulated into the same
psum group). Attention runs
in sim^T layout (keys on partitions, query tokens on the free axis): kT is
pre-scaled by rs_j so softmax is a plain exp; the denominator comes free as a
ones-column appended to v (row 64 of the out psum).

The softmax exp is SPLIT across two engines per j-tile BY HEAD: ACT runs
exact Exp on one head's 512 columns, DVE runs a Schraudolph bf16 exp
(i16 = round(s*A + B) bitcast to bf16, ~3% rel err) on the other head's,
into two separate tiles so the writes carry no ordering edge and each attn.v
matmul waits on exactly one engine. Sides rotate with hg parity so every
token sees Schraudolph on half its heads. The 128-token tail chunk runs as
ONE packed pass (4 head-groups side by side, 512-wide sims/avs).
Normalization: rec = exp(-ln(den)) on ACT straight from the PSUM denominator
row, bf16 PE broadcast, one DVE multiply. x arrives bf16 from the host (the
residual picks up a 0.4% x-rounding, within budget); k and v project in one
[128,n] psum (k rows 0:64, v rows 64:128). The y chain and the LN2 squares
run on the otherwise-idle Pool engine from SBUF-evacuated operands. A 14-
matmul warm-up burst gated on the first x slice flips the PE HAM clock gate
to 2.4 GHz before the projection pipeline starts. The preamble pipelines six
384-token x DMA slices against LN1 stats and the kv projection; deferred
tail phases (out-proj, LN2, y) run one chunk late, each popped BEFORE the
next head-group so they never trail the psum ring.

Precision: bf16 matmul operands with fp32 PSUM accumulation everywhere
(including stat/broadcast matmuls); LN statistics and the residual path stay
fp32. Softmax weights carry the Schraudolph error on DVE tiles, which keeps
the end-to-end absmax rel err ~1e-2 (gate: 2e-2).

Walrus quirks handled: one sync-wait per TPB instruction
(_split_multi_waits); --enable-ldw-opt=true and DVE ALU divide both crash
walrus codegen, so neither is used.
"""

import sys

import numpy as np

if "/opt/trn_rl_repo" not in sys.path:
    sys.path.insert(0, "/opt/trn_rl_repo")

C = 512          # channels
N = 2304         # tokens per batch (48*48)
NH = 1152        # tokens per core
HEADS = 8
DH = 64
CTXL = 77
CTXD = 768
JT = 19          # j tiles of 128: 18 img + 1 (ctx 0:77 | null 77 | pad)
JP = JT * 128
CHUNKS = [(0, 512), (512, 512), (1024, 128)]  # (start, len) token chunks
NCH = len(CHUNKS)
KT = 4           # C / 128
EPS = 1e-5

PROFILE = False
PROFILE_DIR = None

_cached = {}


def _split_multi_waits(nc):
    """Walrus codegen supports one sync-wait per TPB instruction (the EVENTS
    struct has a single wait slot). Tile attaches several. Split the extras
    onto same-engine NoOps inserted just before each instruction."""
    import concourse.mybir as mybir

    n = 0
    for fn in nc.m.functions:
        for bb in fn.blocks:
            insts = bb.instructions
            i = 0
            while i < len(insts):
                ins = insts[i]
                si = getattr(ins, "sync_info", None)
                if si is not None and si.on_wait and len(si.on_wait) > 1:
                    waits = list(si.on_wait)
                    for w in waits[:-1]:
                        n += 1
                        nop = mybir.InstNoOp(name=f"WSPLIT-{n}", engine=ins.engine)
                        nop.sync_info = mybir.SyncInfo(on_wait=[w], on_update=[])
                        insts.insert(i, nop)
                        i += 1
                    ins.sync_info = mybir.SyncInfo(
                        on_wait=[waits[-1]], on_update=si.on_update)
                i += 1
    return n


def _build_bass():
    import concourse.bass as bass
    import concourse.mybir as mybir
    import concourse.tile as tile
    from concourse.masks import make_identity
    from contextlib import ExitStack

    F32 = mybir.dt.float32
    BF = mybir.dt.bfloat16
    I16 = mybir.dt.int16
    AF = mybir.ActivationFunctionType
    ALU = mybir.AluOpType
    # Schraudolph bf16 exp: i16 = round(s*A + B), bitcast to bf16.
    SCHRA_A = float(2 ** 7 / np.log(2.0))
    SCHRA_B = float(127 * 2 ** 7 - 5.0)
    # Softmax exp is split WITHIN each j-tile by head: one head's 512 token
    # columns get exact Exp on ACT, the other head's get Schraudolph on DVE,
    # sides rotating with hg parity so every token sees Schraudolph on
    # exactly half of its heads. The pure per-head split keeps each attn.v
    # matmul dependent on a SINGLE exp engine (a mid-head split chains every
    # av onto ACT completion and serializes the loop).

    nc = bass.Bass()
    x_own = nc.declare_dram_parameter("x_own", [C, NH], BF, isOutput=False)
    x_oth = nc.declare_dram_parameter("x_oth", [C, NH], BF, isOutput=False)
    ctxt = nc.declare_dram_parameter("ctxt", [CTXL, CTXD], F32, isOutput=False)
    wq = nc.declare_dram_parameter("wq", [C, C], BF, isOutput=False)
    negcq = nc.declare_dram_parameter("negcq", [1, C], BF, isOutput=False)
    wkv = nc.declare_dram_parameter("wkv", [C, 2 * DH], BF, isOutput=False)
    ncskv = nc.declare_dram_parameter("ncskv", [1, 2 * DH], BF, isOutput=False)
    wctx = nc.declare_dram_parameter("wctx", [CTXD, 2 * DH], F32, isOutput=False)
    bctxk = nc.declare_dram_parameter("bctxk", [DH, 1], F32, isOutput=False)
    bctxv = nc.declare_dram_parameter("bctxv", [DH, 1], F32, isOutput=False)
    nullkt = nc.declare_dram_parameter("nullkt", [DH, 1], F32, isOutput=False)
    nullv = nc.declare_dram_parameter("nullv", [DH, 1], F32, isOutput=False)
    wout = nc.declare_dram_parameter("wout", [DH, HEADS * C], BF, isOutput=False)
    outg = nc.declare_dram_parameter("outg", [128, KT], F32, isOutput=False)
    y = nc.declare_dram_parameter("y", [C, NH], F32, isOutput=True)

    with tile.TileContext(nc) as tc, ExitStack() as ctx:
        pconst = ctx.enter_context(tc.tile_pool(name="const", bufs=1))
        pbig = ctx.enter_context(tc.tile_pool(name="big", bufs=1))

        ident = pconst.tile([128, 128], F32)
        make_identity(nc, ident[:])
        ident_bf = pconst.tile([128, 128], BF)
        make_identity(nc, ident_bf[:])
        # 64x64 identity living on partitions 64:128 (moving operand for the
        # v transpose, whose stationary input sits on the upper partitions)
        ident_hi = pconst.tile([128, 64], BF)
        make_identity(nc, ident_hi[64:128, 0:64])
        ones_col = pconst.tile([128, 1], F32)
        nc.vector.memset(ones_col[:], 1.0)
        ones_blk_bf = pconst.tile([128, 128], BF)
        nc.vector.memset(ones_blk_bf[:], 1.0)
        eps_col = pconst.tile([128, 1], F32)
        nc.vector.memset(eps_col[:], EPS)

        x_bf = pbig.tile([128, KT * N], BF)          # kt-major; own rows first
        # outg broadcast to full planes once, so the Pool-engine y chain
        # needs only tensor_tensor ops (Pool tensor_scalar is ~6x slower)
        outg_pl = pbig.tile([128, KT * 512], F32)
        # head-pair blocks + a packed block for the 128-token tail chunk
        # (all 4 hgs' last-128 tokens side by side -> full-width matmuls)
        QP = (HEADS // 2) * NH
        qT = pbig.tile([128, QP + 512], BF)
        kT2 = pbig.tile([128, JP], BF)               # rs-scaled keys, both halves
        v_sb = pbig.tile([128, JT * (DH + 1)], BF)   # per j-tile [128, 64+ones]
        projT = pbig.tile([128, KT * NH], BF)
        stats = pbig.tile([128, 40], F32)            # col jt: rs_j (v scaling)
        wout_sb = pbig.tile([64, HEADS * C], BF)
        outg_sb = pbig.tile([128, KT], F32)
        # per-token stat rows on partition 0: mu 0:N | rs N:2N
        # (LN2 reuses per cc: mu2 at cc*CH, rs2 at N+cc*CH, ex2 at 2N+cc*CH)
        rows = pbig.tile([1, 2 * N + NH], F32)
        rows_bf = pbig.tile([1, N], BF)
        R_RS, R_SC = N, 2 * N



        with tc.tile_pool(name="load", bufs=1) as pload, \
             tc.tile_pool(name="x2p", bufs=2) as px2, \
             tc.tile_pool(name="pss", bufs=2, space="PSUM") as pss:
            # setup psum tags: b1 [<=64,384]x2, bS [128,<=512]x4, bT [128,128]x2
            wq_sb = pload.tile([128, KT * C], BF)
            wkv_sb = pload.tile([128, KT * 2 * DH], BF)
            wctx_sb = pload.tile([128, CTXD], F32)
            negcq_sb = pload.tile([1, C], BF)
            ncskv_sb = pload.tile([1, 2 * DH], BF)
            bctxk_sb = pload.tile([DH, 1], F32)
            bctxv_sb = pload.tile([DH, 1], F32)
            vT = pload.tile([128, N], BF)     # v rows live at partitions 64:128
            ck_sb = pload.tile([64, CTXL], F32)
            cv_sb = pload.tile([64, CTXL + 1], F32)
            nullk_st = pload.tile([DH, 1], F32)
            nullv_st = pload.tile([DH, 1], F32)
            ctx_sb = pload.tile([CTXL, CTXD], F32)
            ctxnT = pload.tile([128, 6 * CTXL], F32)
            ex2 = pload.tile([1, N], F32)

            x_v = x_bf[:].rearrange("p (k n) -> p k n", k=KT)
            # Three DGE queues (SP/Pool/ACT). Queue heads carry what unblocks
            # compute first: x slice 0 (stats pipeline), ctx (LN chain), wctx
            # (ctx projection). Weights follow behind the first x slices.
            xo = [x_own[:, a : a + 384].rearrange("(k p) n -> p k n", p=128)
                  for a in (0, 384, 768)]
            xt = [x_oth[:, a : a + 384].rearrange("(k p) n -> p k n", p=128)
                  for a in (0, 384, 768)]
            nc.sync.dma_start(x_v[:, :, 0:384], xo[0])
            nc.gpsimd.dma_start(ctx_sb[:], ctxt[:, :])
            nc.gpsimd.dma_start(x_v[:, :, 384:768], xo[1])
            nc.scalar.dma_start(wctx_sb[:].rearrange("p (k n) -> p k n", k=6),
                                wctx[:].rearrange("(k p) n -> p k n", p=128))
            nc.scalar.dma_start(x_v[:, :, 768:1152], xo[2])
            nc.sync.dma_start(wkv_sb[:].rearrange("p (k n) -> p k n", k=KT),
                              wkv[:].rearrange("(k p) n -> p k n", p=128))
            nc.sync.dma_start(ncskv_sb[:], ncskv[:, :])
            nc.sync.dma_start(wq_sb[:].rearrange("p (k n) -> p k n", k=KT),
                              wq[:].rearrange("(k p) n -> p k n", p=128))
            nc.sync.dma_start(negcq_sb[:], negcq[:, :])
            nc.gpsimd.dma_start(nullk_st[:], nullkt[:, :])
            nc.gpsimd.dma_start(nullv_st[:], nullv[:, :])
            nc.scalar.dma_start(bctxk_sb[:], bctxk[:, :])
            nc.scalar.dma_start(bctxv_sb[:], bctxv[:, :])
            nc.sync.dma_start(x_v[:, :, NH : NH + 384], xt[0])
            nc.gpsimd.dma_start(x_v[:, :, NH + 384 : NH + 768], xt[1])
            nc.scalar.dma_start(x_v[:, :, NH + 768 : NH + 1152], xt[2])
            nc.scalar.dma_start(wout_sb[:], wout[:, :])
            nc.scalar.dma_start(outg_sb[:], outg[:, :])

            # ---- context: LN (layout A, bn_stats) + k/v projection ----
            cstat = pload.tile([CTXL, 3, 6], F32)
            for sg in range(3):
                nc.vector.bn_stats(cstat[:, sg, :],
                                   ctx_sb[:, sg * 256 : (sg + 1) * 256])
            cmv = pload.tile([CTXL, 2], F32)
            nc.vector.bn_aggr(cmv[:], cstat[:])
            nc.scalar.activation(cmv[:, 1:2], cmv[:, 1:2], AF.Ln,
                                 bias=eps_col[0:CTXL, :])
            nc.scalar.activation(cmv[:, 1:2], cmv[:, 1:2], AF.Exp, scale=-0.5)
            nc.vector.tensor_scalar(
                out=ctx_sb[:], in0=ctx_sb[:],
                scalar1=cmv[:, 0:1], scalar2=cmv[:, 1:2],
                op0=ALU.subtract, op1=ALU.mult)
            for kt in range(6):
                ps_ct = pss.tile([128, 128], F32, tag="bT")
                nc.tensor.transpose(ps_ct[:, 0:CTXL],
                                    ctx_sb[:, kt * 128 : (kt + 1) * 128],
                                    ident[:CTXL, :CTXL])
                nc.vector.tensor_copy(ctxnT[:, kt * CTXL : (kt + 1) * CTXL],
                                      ps_ct[:, 0:CTXL])
            ps_ck = pss.tile([64, 384], F32, tag="b1")
            ps_cv = pss.tile([64, 384], F32, tag="b1")
            for kt in range(6):
                nc.tensor.matmul(ps_ck[:, 0:CTXL],
                                 wctx_sb[:, kt * 128 : kt * 128 + DH],
                                 ctxnT[:, kt * CTXL : (kt + 1) * CTXL],
                                 start=(kt == 0), stop=(kt == 5))
                nc.tensor.matmul(ps_cv[:, 0:CTXL],
                                 wctx_sb[:, kt * 128 + DH : (kt + 1) * 128],
                                 ctxnT[:, kt * CTXL : (kt + 1) * CTXL],
                                 start=(kt == 0), stop=(kt == 5))
            # ---- PE clock warm-up: HAM throttles the PE to 1.2 GHz until it
            # sees ~3.4us of sustained activity. Burn fp32 matmuls on the
            # first x slice (dep => they start right as x lands, ~14us) so
            # the real projection pipeline runs at 2.4 GHz from the start.
            for _ in range(14):
                ps_w = pss.tile([1, 384], F32, tag="warm")
                nc.tensor.matmul(ps_w[0:1, :], ones_blk_bf[:, 0:1],
                                 x_bf[:, 0:384], start=True, stop=True)

            nc.vector.tensor_scalar_add(ck_sb[:], ps_ck[:, 0:CTXL], bctxk_sb[:])
            nc.vector.tensor_scalar_add(cv_sb[:, 0:CTXL], ps_cv[:, 0:CTXL],
                                        bctxv_sb[:])
            nc.vector.tensor_copy(cv_sb[:, CTXL : CTXL + 1], nullv_st[:])

            # ---- j-tile 18: [ctx 0:77 | null 77 | pad 78:128] ----
            VB = 18 * (DH + 1)
            nc.vector.memset(kT2[0:64, 18 * 128 : JP], 0.0)
            nc.vector.tensor_copy(kT2[0:64, 18 * 128 : 18 * 128 + CTXL], ck_sb[:])
            nc.vector.tensor_copy(kT2[0:64, 18 * 128 + CTXL : 18 * 128 + CTXL + 1],
                                  nullk_st[:])
            nc.vector.memset(v_sb[:, VB : VB + DH + 1], 0.0)
            ps_cvt = pss.tile([128, 128], F32, tag="bT")
            nc.tensor.transpose(ps_cvt[0 : CTXL + 1, 0:64], cv_sb[:],
                                ident[:64, :64])
            nc.vector.tensor_copy(v_sb[0 : CTXL + 1, VB : VB + DH],
                                  ps_cvt[0 : CTXL + 1, 0:64])
            nc.vector.memset(v_sb[0 : CTXL + 1, VB + DH : VB + DH + 1], 1.0)
            nc.gpsimd.dma_start(kT2[64:128, 18 * 128 : JP],
                                kT2[0:64, 18 * 128 : JP])

            # ---- per-384-token pipeline: stats -> kv proj -> v tiles ----
            # (each stage consumes one x DMA slice as it lands)
            ones_col_bf = pconst.tile([128, 1], BF)
            nc.vector.memset(ones_col_bf[:], 1.0)
            rs_bf = pload.tile([1, N], BF)

            def qproj(a0, ln, packed=False):
                # q projection for one token chunk (LN + 1/sqrt(dh) folded);
                # interleaved into the per-384 pipeline as soon as its token
                # span has stats, so q DVE work doesn't queue behind all kv.
                # packed=True lays the 4 head-groups side by side at QP
                # (tail chunk: one 512-wide attention pass over all 8 heads)
                sl = slice(a0, a0 + ln)
                ps_rs = pss.tile([128, 512], F32, tag="bS")
                nc.tensor.matmul(ps_rs[:, 0:ln], ones_blk_bf[0:1, :],
                                 rs_bf[0:1, sl])
                rs_b = px2.tile([128, 512], F32, tag="rsb")
                nc.vector.tensor_copy(rs_b[:, 0:ln], ps_rs[:, 0:ln])
                for hg in range(HEADS // 2):
                    ps_q = pss.tile([128, 512], F32, tag="bS")
                    for kt in range(KT):
                        nc.tensor.matmul(
                            ps_q[:, 0:ln],
                            wq_sb[:, kt * C + hg * 128 : kt * C + (hg + 1) * 128],
                            x_bf[:, kt * N + a0 : kt * N + a0 + ln],
                            start=(kt == 0), stop=False)
                    nc.tensor.matmul(ps_q[:, 0:ln],
                                     negcq_sb[0:1, hg * 128 : (hg + 1) * 128],
                                     rows_bf[0:1, sl], start=False, stop=True)
                    d0 = QP + hg * 128 if packed else hg * NH + a0
                    nc.vector.tensor_mul(
                        qT[:, d0 : d0 + ln],
                        ps_q[:, 0:ln], rs_b[:, 0:ln])

            for ch in range(6):
                sl = slice(ch * 384, (ch + 1) * 384)
                ps_r1 = pss.tile([64, 384], F32, tag="b1")
                for kt in range(KT):
                    nc.tensor.matmul(
                        ps_r1[0:1, :], ones_col_bf[:],
                        x_bf[:, kt * N + ch * 384 : kt * N + (ch + 1) * 384],
                        start=(kt == 0), stop=(kt == KT - 1))
                nc.scalar.mul(rows[0:1, sl], ps_r1[0:1, :], 1.0 / C)
                nc.vector.tensor_copy(rows_bf[0:1, sl], rows[0:1, sl])
                x2 = px2.tile([128, KT * 384], BF, tag="x2")
                ps_r2 = pss.tile([64, 384], F32, tag="b1")
                for kt in range(KT):
                    xs = x_bf[:, kt * N + ch * 384 : kt * N + (ch + 1) * 384]
                    nc.gpsimd.tensor_mul(x2[:, kt * 384 : (kt + 1) * 384], xs, xs)
                    nc.tensor.matmul(
                        ps_r2[0:1, :], ones_col_bf[:],
                        x2[:, kt * 384 : (kt + 1) * 384],
                        start=(kt == 0), stop=(kt == KT - 1))
                nc.scalar.mul(ex2[0:1, ch * 384 : (ch + 1) * 384],
                              ps_r2[0:1, :], 1.0 / C)
                a, b = R_RS + ch * 384, R_RS + (ch + 1) * 384
                mu = rows[0:1, ch * 384 : (ch + 1) * 384]
                nc.vector.tensor_mul(rows[0:1, a:b], mu, mu)
                nc.vector.tensor_sub(rows[0:1, a:b],
                                     ex2[0:1, ch * 384 : (ch + 1) * 384],
                                     rows[0:1, a:b])
                nc.scalar.activation(rows[0:1, a:b], rows[0:1, a:b], AF.Ln,
                                     bias=eps_col[0:1, :])
                nc.scalar.activation(rows[0:1, a:b], rows[0:1, a:b], AF.Exp,
                                     scale=-0.5)
                nc.vector.tensor_copy(
                    rs_bf[0:1, sl],
                    rows[0:1, R_RS + ch * 384 : R_RS + (ch + 1) * 384])
                # rs as per-partition columns (v scaling), 3 j-tiles per ch
                for jt in range(ch * 3, ch * 3 + 3):
                    ps_c = pss.tile([128, 128], F32, tag="bT")
                    nc.tensor.matmul(
                        ps_c[:, 0:1],
                        rows[0:1, R_RS + jt * 128 : R_RS + (jt + 1) * 128],
                        ones_col[0:1, :])
                    nc.vector.tensor_copy(stats[:, jt : jt + 1], ps_c[:, 0:1])
                # kv projection for this 384-token slice (LN folded): k and
                # v land in one [128, n] psum (k rows 0:64, v rows 64:128)
                kva, kvl = ch * 384, 384
                ps_kv = pss.tile([128, 512], F32, tag="bS")
                for kt in range(KT):
                    xs = x_bf[:, kt * N + kva : kt * N + kva + kvl]
                    nc.tensor.matmul(ps_kv[0:128, 0:kvl],
                                     wkv_sb[:, kt * 128 : (kt + 1) * 128],
                                     xs, start=(kt == 0), stop=False)
                nc.tensor.matmul(ps_kv[0:128, 0:kvl], ncskv_sb[:],
                                 rows_bf[0:1, sl], start=False, stop=True)
                ps_bc = pss.tile([128, 512], F32, tag="bS")
                nc.tensor.matmul(ps_bc[0:64, 0:kvl], ones_blk_bf[0:1, 0:64],
                                 rs_bf[0:1, sl])
                kk = px2.tile([64, 512], F32, tag="kk")
                nc.vector.tensor_copy(kk[:, 0:kvl], ps_kv[0:64, 0:kvl])
                nc.vector.tensor_mul(kT2[0:64, sl], kk[:, 0:kvl],
                                     ps_bc[0:64, 0:kvl])
                # duplicate this slice to partitions 64:128 right away, so
                # attention isn't gated on the LAST kv slice + one big copy
                nc.sync.dma_start(kT2[64:128, sl], kT2[0:64, sl])
                nc.vector.tensor_copy(vT[64:128, sl], ps_kv[64:128, 0:kvl])
                # v tiles: transpose + rs scale + ones col
                for jt in range(ch * 3, ch * 3 + 3):
                    ps_vt = pss.tile([128, 128], BF, tag="bT")
                    nc.tensor.transpose(ps_vt[:, 0:64],
                                        vT[64:128, jt * 128 : (jt + 1) * 128],
                                        ident_hi[64:128, 0:64])
                    vb = jt * (DH + 1)
                    nc.vector.tensor_scalar_mul(v_sb[:, vb : vb + DH],
                                                ps_vt[:, 0:64],
                                                stats[:, jt : jt + 1])
                    nc.vector.memset(v_sb[:, vb + DH : vb + DH + 1], 1.0)
                if ch == 1:
                    qproj(0, 512)
                elif ch == 2:
                    qproj(512, 512)
                    qproj(1024, 128, packed=True)

            # outg broadcast planes for the Pool-engine y chain (placed after
            # the pipeline so the in-order DVE queue never stalls on the late
            # outg DMA)
            nc.vector.memset(outg_pl[:, 0:512], 1.0)
            for ct in range(1, KT):
                nc.vector.tensor_copy(outg_pl[:, ct * 512 : (ct + 1) * 512],
                                      outg_pl[:, 0:512])
            for ct in range(KT):
                nc.vector.tensor_scalar_mul(
                    outg_pl[:, ct * 512 : (ct + 1) * 512],
                    outg_pl[:, ct * 512 : (ct + 1) * 512],
                    outg_sb[:, ct : ct + 1])

        # ========= attention + output + LN2 + residual, per chunk =========
        # Per (chunk, head-pair): row-packed sims -> exp (split ACT/DVE by
        # j-tile: ACT exact, DVE Schraudolph-bf16) -> attn.v pair lagging one
        # j-tile. Softmax normalization: rec = exp(-ln(den)) on ACT straight
        # from PSUM, bf16 PE broadcast, one DVE mul -> ot. The output tail
        # (out-proj, LN2, y) is DEFERRED one chunk so tail matmuls never
        # head-of-line-block the in-order PE queue.
        with tc.tile_pool(name="attn", bufs=6) as pattn, \
             tc.tile_pool(name="outp", bufs=16) as pout, \
             tc.tile_pool(name="recp", bufs=6) as prec, \
             tc.tile_pool(name="rbsp", bufs=4) as prbs, \
             tc.tile_pool(name="p2p", bufs=2) as pp2, \
             tc.tile_pool(name="yp", bufs=4) as pyt, \
             tc.tile_pool(name="rwbp", bufs=2) as prwb, \
             tc.tile_pool(name="psatt", bufs=2, space="PSUM") as psA, \
             tc.tile_pool(name="pspo", bufs=3, space="PSUM") as psB, \
             tc.tile_pool(name="psmx", bufs=1, space="PSUM") as psC:

            def exp_split(ps0, ps1, act_first):
                # one head's sim tile gets exact Exp on ACT, the other
                # Schraudolph on DVE. Separate PSUM sim tiles AND separate
                # output tiles per engine: any shared tile (even disjoint
                # halves) gives the dep tracker an ACT->DVE edge that
                # serializes the two exps. Returns (mv0, mv1) moving APs
                # for the two attn.v matmuls.
                atA = pattn.tile([128, 512], BF, tag="atA")
                atB = pattn.tile([128, 512], I16, tag="atB")
                psa, psd = (ps0, ps1) if act_first else (ps1, ps0)
                nc.scalar.activation(atA[:], psa[:, 0:512], AF.Exp)
                nc.vector.tensor_scalar(
                    out=atB[:], in0=psd[:, 0:512],
                    scalar1=SCHRA_A, scalar2=SCHRA_B,
                    op0=ALU.mult, op1=ALU.add)
                if act_first:
                    return atA[:], atB[:].bitcast(BF)
                return atB[:].bitcast(BF), atA[:]

            def normalize(po, ln, ots):
                # rec = exp(-ln(den)) on ACT (reads the PSUM denominator
                # row), bf16 PE broadcast, one DVE mul -> ot
                recb = prec.tile([65, 512], F32, tag="rec")
                nc.scalar.activation(recb[64:65, 0:ln], po[64:65, 0:ln],
                                     AF.Ln, bias=eps_col[64:65, :])
                rec_bf = prec.tile([65, 512], BF, tag="recb")
                nc.scalar.activation(rec_bf[64:65, 0:ln],
                                     recb[64:65, 0:ln], AF.Exp, scale=-1.0)
                ps_rb = psC.tile([128, 512], F32, tag="mx")
                nc.tensor.matmul(ps_rb[0:64, 0:ln],
                                 ones_blk_bf[64:65, 0:64],
                                 rec_bf[64:65, 0:ln],
                                 start=True, stop=True)
                rb_sb = prbs.tile([64, 512], F32, tag="rbs")
                nc.vector.tensor_copy(rb_sb[:, 0:ln], ps_rb[0:64, 0:ln])
                ot = pout.tile([64, 512], BF, tag="ot")
                nc.vector.tensor_mul(ot[:, 0:ln], po[0:64, 0:ln],
                                     rb_sb[:, 0:ln])
                ots.append(ot)

            def run_hg(q0, q1, ln, ots, hg):
                po0 = psB.tile([65, 512], F32, tag="po")
                po1 = psB.tile([65, 512], F32, tag="po")
                po = [po0, po1]
                off1 = 512
                ats = [None] * JT
                for jt in range(JT):
                    ps0 = psA.tile([128, 512], F32, tag="simA")
                    ps1 = psA.tile([128, 512], F32, tag="simB")
                    nc.tensor.matmul(ps0[:, 0:ln],
                                     kT2[0:64, jt * 128 : (jt + 1) * 128],
                                     q0, start=True, stop=True)
                    nc.tensor.matmul(ps1[:, 0:ln],
                                     kT2[64:128, jt * 128 : (jt + 1) * 128],
                                     q1, start=True, stop=True)
                    # ln == 512 here (tail chunk runs packed in run_tail)
                    ats[jt] = exp_split(ps0, ps1, act_first=(hg % 2 == 0))
                    if jt > 0:
                        j0 = jt - 1
                        vs = v_sb[:, j0 * (DH + 1) : (j0 + 1) * (DH + 1)]
                        nc.tensor.matmul(po[0][0:65, 0:ln], vs,
                                         ats[j0][0][:, 0:ln],
                                         start=(j0 == 0), stop=False)
                        nc.tensor.matmul(po[1][0:65, 0:ln], vs,
                                         ats[j0][1][:, 0:ln],
                                         start=(j0 == 0), stop=False)
                        ats[j0] = None
                j0 = JT - 1
                vs = v_sb[:, j0 * (DH + 1) : (j0 + 1) * (DH + 1)]
                nc.tensor.matmul(po[0][0:65, 0:ln], vs,
                                 ats[j0][0][:, 0:ln],
                                 start=False, stop=True)
                nc.tensor.matmul(po[1][0:65, 0:ln], vs,
                                 ats[j0][1][:, 0:ln],
                                 start=False, stop=True)
                for i in range(2):
                    normalize(po[i], ln, ots)

            def run_tail(phases):
                # tail chunk (tokens 1024:1152): ONE pass over the j-tiles
                # with all 4 head-groups packed side by side (128 cols each):
                # cols 0:512 first-of-pair heads, 512:1024 second-of-pair.
                ots = []
                poA = psB.tile([65, 512], F32, tag="po")
                poB = psB.tile([65, 512], F32, tag="po")
                ats = [None] * JT
                for jt in range(JT):
                    # pop only mx-backed phases (proj/ln2) inside the pass;
                    # the y phase touches the po ring poA/poB live on
                    if len(phases) > 1 and jt in (2, 5, 8, 11, 14):
                        phases.pop(0)()
                    ps0 = psA.tile([128, 512], F32, tag="simA")
                    ps1 = psA.tile([128, 512], F32, tag="simB")
                    nc.tensor.matmul(ps0[:, 0:512],
                                     kT2[0:64, jt * 128 : (jt + 1) * 128],
                                     qT[0:64, QP : QP + 512],
                                     start=True, stop=True)
                    nc.tensor.matmul(ps1[:, 0:512],
                                     kT2[64:128, jt * 128 : (jt + 1) * 128],
                                     qT[64:128, QP : QP + 512],
                                     start=True, stop=True)
                    ats[jt] = exp_split(ps0, ps1, act_first=True)
                    if jt > 0:
                        j0 = jt - 1
                        vs = v_sb[:, j0 * (DH + 1) : (j0 + 1) * (DH + 1)]
                        nc.tensor.matmul(poA[0:65, 0:512], vs,
                                         ats[j0][0][:, 0:512],
                                         start=(j0 == 0), stop=False)
                        nc.tensor.matmul(poB[0:65, 0:512], vs,
                                         ats[j0][1][:, 0:512],
                                         start=(j0 == 0), stop=False)
                        ats[j0] = None
                j0 = JT - 1
                vs = v_sb[:, j0 * (DH + 1) : (j0 + 1) * (DH + 1)]
                nc.tensor.matmul(poA[0:65, 0:512], vs,
                                 ats[j0][0][:, 0:512],
                                 start=False, stop=True)
                nc.tensor.matmul(poB[0:65, 0:512], vs,
                                 ats[j0][1][:, 0:512],
                                 start=False, stop=True)
                normalize(poA, 512, ots)
                normalize(poB, 512, ots)
                return ots

            def tail_phases(a0, ln, ots, packed=False, last=False):
                def ph_proj(ct):
                    ps_p = psC.tile([128, 512], F32, tag="mx")
                    for h in range(HEADS):
                        if packed:
                            mv = ots[h % 2][:, (h // 2) * 128 :
                                            (h // 2) * 128 + 128]
                        else:
                            mv = ots[h][:, 0:ln]
                        nc.tensor.matmul(
                            ps_p[:, 0:ln],
                            wout_sb[:, h * C + ct * 128 : h * C + (ct + 1) * 128],
                            mv,
                            start=(h == 0), stop=(h == HEADS - 1))
                    nc.vector.tensor_copy(
                        projT[:, ct * NH + a0 : ct * NH + a0 + ln],
                        ps_p[:, 0:ln])

                def ph_ln2():
                    ra, rb2 = R_RS + a0, R_RS + a0 + ln
                    sca, scb = R_SC + a0, R_SC + a0 + ln
                    ps_m2 = psC.tile([128, 512], F32, tag="mx")
                    for ct in range(KT):
                        nc.tensor.matmul(
                            ps_m2[0:1, 0:ln], ones_col_bf[:],
                            projT[:, ct * NH + a0 : ct * NH + a0 + ln],
                            start=(ct == 0), stop=(ct == KT - 1))
                    nc.scalar.mul(rows[0:1, a0 : a0 + ln], ps_m2[0:1, 0:ln], 1.0 / C)
                    p2 = pp2.tile([128, KT * 512], BF, tag="p2")
                    ps_q2 = psC.tile([128, 512], F32, tag="mx")
                    sq_eng = nc.vector if last else nc.gpsimd
                    for ct in range(KT):
                        pslc = projT[:, ct * NH + a0 : ct * NH + a0 + ln]
                        sq_eng.tensor_mul(p2[:, ct * 512 : ct * 512 + ln],
                                          pslc, pslc)
                        nc.tensor.matmul(ps_q2[0:1, 0:ln], ones_col_bf[:],
                                         p2[:, ct * 512 : ct * 512 + ln],
                                         start=(ct == 0), stop=(ct == KT - 1))
                    nc.scalar.mul(rows[0:1, sca:scb], ps_q2[0:1, 0:ln], 1.0 / C)
                    nc.vector.tensor_mul(rows[0:1, ra:rb2], rows[0:1, a0 : a0 + ln],
                                         rows[0:1, a0 : a0 + ln])
                    nc.vector.tensor_sub(rows[0:1, ra:rb2], rows[0:1, sca:scb],
                                         rows[0:1, ra:rb2])
                    nc.scalar.activation(rows[0:1, ra:rb2], rows[0:1, ra:rb2],
                                         AF.Ln, bias=eps_col[0:1, :])
                    nc.scalar.activation(rows[0:1, ra:rb2], rows[0:1, ra:rb2],
                                         AF.Exp, scale=-0.5)
                    rwb = prwb.tile([1, 1024], BF, tag="rwb")
                    nc.vector.tensor_copy(rwb[0:1, 0:ln], rows[0:1, a0 : a0 + ln])
                    nc.vector.tensor_copy(rwb[0:1, 512 : 512 + ln],
                                          rows[0:1, ra:rb2])
                    return rwb

                def ph_y(rwb):
                    ps_bm = psB.tile([128, 512], F32, tag="po")
                    nc.tensor.matmul(ps_bm[:, 0:ln], ones_blk_bf[0:1, :],
                                     rwb[0:1, 0:ln], start=True, stop=True)
                    ps_br = psB.tile([128, 512], F32, tag="po")
                    nc.tensor.matmul(ps_br[:, 0:ln], ones_blk_bf[0:1, :],
                                     rwb[0:1, 512 : 512 + ln],
                                     start=True, stop=True)
                    # evacuate broadcasts to SBUF so the y chain can run on
                    # the (otherwise idle) Pool engine, which can't read PSUM
                    bmr = pyt.tile([128, 1024], F32, tag="bmr")
                    nc.vector.tensor_copy(bmr[:, 0:ln], ps_bm[:, 0:ln])
                    nc.vector.tensor_copy(bmr[:, 512 : 512 + ln],
                                          ps_br[:, 0:ln])
                    for ct in range(KT):
                        # during attention the y chain runs on the idle Pool
                        # engine; for the final chunk (nothing left to
                        # overlap) split it across DVE and Pool for latency
                        eng = (nc.vector if ct % 2 == 0 else
                               nc.gpsimd) if last else nc.gpsimd
                        yt = pyt.tile([128, 512], F32, tag="yt")
                        pslice = projT[:, ct * NH + a0 : ct * NH + a0 + ln]
                        eng.tensor_sub(yt[:, 0:ln], pslice, bmr[:, 0:ln])
                        eng.tensor_mul(yt[:, 0:ln], yt[:, 0:ln],
                                       bmr[:, 512 : 512 + ln])
                        eng.tensor_mul(yt[:, 0:ln], yt[:, 0:ln],
                                       outg_pl[:, ct * 512 : ct * 512 + ln])
                        eng.tensor_add(
                            yt[:, 0:ln], yt[:, 0:ln],
                            x_bf[:, ct * N + a0 : ct * N + a0 + ln])
                        nc.sync.dma_start(
                            y[ct * 128 : (ct + 1) * 128, a0 : a0 + ln],
                            yt[:, 0:ln])

                state = {}

                def s_ln2():
                    state["rwb"] = ph_ln2()

                def s_y():
                    ph_y(state["rwb"])

                # six phases: 4 per-ct out-projections (PSUM tag mx only —
                # safe to pop anywhere), ln2 (mx), y (takes po-ring slots:
                # only safe to pop at head-group boundaries, never mid-loop)
                return [lambda ct=ct: ph_proj(ct) for ct in range(KT)] + \
                    [s_ln2, s_y]

            phases = []
            for a0, ln in CHUNKS[:2]:
                ots = []
                for hg in range(HEADS // 2):
                    # pop TWO phases at boundary slots 1..3 (slot 0 would
                    # make ph_proj wait on the previous chunk's final
                    # head-group normalization); mid-loop injection stalls
                    # the attention pipeline, so boundaries only
                    if phases and hg > 0:
                        phases.pop(0)()
                        if phases:
                            phases.pop(0)()
                    run_hg(qT[0:64, hg * NH + a0 : hg * NH + a0 + ln],
                           qT[64:128, hg * NH + a0 : hg * NH + a0 + ln],
                           ln, ots, hg)
                phases = tail_phases(a0, ln, ots)
            ots = run_tail(phases)
            # remaining phase of chunk 1 (y) overlaps the tail's own phases
            for ph in phases:
                ph()
            phases = tail_phases(1024, 128, ots, packed=True, last=True)
            for ph in phases:
                ph()
    _split_multi_waits(nc)
    return nc


def _prep_inputs(x, context, norm_gamma, null_kv, Wq, Wkv, ctx_ln_g, ctx_ln_b,
                 Wctx, bctx, Wout, out_ln_g):
    import ml_dtypes
    bf = ml_dtypes.bfloat16
    f = np.float32
    x = np.asarray(x, f).reshape(4, C, N)
    context = np.asarray(context, f)
    g = np.asarray(norm_gamma, f)
    scale = 1.0 / np.sqrt(DH)
    wq_h = (g[:, None] * np.asarray(Wq, f)) * scale
    negcq_h = -wq_h.sum(0, dtype=np.float64).astype(f)[None, :]
    wkv_h = g[:, None] * np.asarray(Wkv, f)
    ncsk_h = -wkv_h[:, :DH].sum(0, dtype=np.float64).astype(f)[None, :]
    ncsv_h = -wkv_h[:, DH:].sum(0, dtype=np.float64).astype(f)[None, :]
    wctx_h = np.asarray(ctx_ln_g, f)[:, None] * np.asarray(Wctx, f)
    bctx_h = (np.asarray(bctx, f) + np.asarray(ctx_ln_b, f) @ np.asarray(Wctx, f))
    null = np.asarray(null_kv, f)
    wout_b = np.concatenate(
        [np.asarray(Wout, f)[h * DH:(h + 1) * DH, :] for h in range(HEADS)], axis=1)
    outg_h = np.ascontiguousarray(np.asarray(out_ln_g, f).reshape(KT, 128).T)

    shared = {
        "wq": np.ascontiguousarray(wq_h).astype(bf),
        "negcq": negcq_h.astype(bf),
        "wkv": np.ascontiguousarray(wkv_h).astype(bf),
        "ncskv": np.concatenate([ncsk_h, ncsv_h], axis=1).astype(bf),
        "wctx": np.ascontiguousarray(wctx_h),
        "bctxk": np.ascontiguousarray(bctx_h[:DH, None]),
        "bctxv": np.ascontiguousarray(bctx_h[DH:, None]),
        "nullkt": np.ascontiguousarray(null[0][:, None]),
        "nullv": np.ascontiguousarray(null[1][:, None]),
        "wout": np.ascontiguousarray(wout_b).astype(bf),
        "outg": outg_h,
    }
    in_maps = []
    for core in range(8):
        b, half = core // 2, core % 2
        m = dict(shared)
        m["x_own"] = np.ascontiguousarray(
            x[b][:, half * NH : (half + 1) * NH]).astype(bf)
        m["x_oth"] = np.ascontiguousarray(
            x[b][:, (1 - half) * NH : (2 - half) * NH]).astype(bf)
        m["ctxt"] = np.ascontiguousarray(context[b])
        in_maps.append(m)
    return in_maps


def _register_profile_hook():
    if "antenv.axon_hooks" in sys.modules:
        return
    import types

    m = types.ModuleType("antenv.axon_hooks")
    m._hook = None

    def _set_hook(h, _m=m):
        _m._hook = h

    def _get_hook(_m=m):
        return _m._hook

    m.set_axon_ntff_profile_hook = _set_hook
    m.get_axon_ntff_profile_hook = _get_hook
    sys.modules["antenv.axon_hooks"] = m
    import antenv

    antenv.axon_hooks = m
    from trn_agent_boot.trn_boot import _ntff_profile_via_ctypes

    _set_hook(_ntff_profile_via_ctypes("/opt/axon/libaxon_pjrt.so"))


_LDW_OPT = [False]


def _patch_ldw_opt():
    import concourse.bass_utils as bu
    if getattr(bu, "_ldwopt_patched", False):
        return
    orig = bu.run_command

    def run2(cmd, **kw):
        if _LDW_OPT[0]:
            cmd = [c.replace("--enable-ldw-opt=false", "--enable-ldw-opt=true")
                   for c in cmd]
        return orig(cmd, **kw)

    bu.run_command = run2
    bu._ldwopt_patched = True


def kernel(**inputs):
    from concourse.bass_utils import run_bass_kernel_spmd
    _patch_ldw_opt()

    if "nc" not in _cached:
        _cached["nc"] = _build_bass()
    nc = _cached["nc"]
    in_maps = _prep_inputs(**inputs)
    kw = {}
    if PROFILE:
        _register_profile_hook()
        kw = dict(trace=True, tmpdir=PROFILE_DIR)
    res = run_bass_kernel_spmd(nc, in_maps, list(range(8)), **kw)
    _cached["last"] = res
    out = np.empty((4, C, N), np.float32)
    for core in range(8):
        b, half = core // 2, core % 2
        out[b][:, half * NH : (half + 1) * NH] = res.results[core]["y"]
    return out.reshape(4, C, 48, 48)



# revision 60
# speedup vs baseline: 1.1167x; 1.1167x over previous
"""Trainium2 Bass kernel for nn_Attention_LR_65249143160949 (cross-attention block).

Sharding: 8 cores = 4 batches x 2 token-halves (1152 tokens each). Each core
computes k/v for its whole batch (cheap MQA single head, duplicated within the
pair) and q/attention/output for its own tokens. The host permutes tokens so
each core's own rows come first -> identical SPMD program, no collectives.

On-chip layout: features on partitions, tokens on the free axis (matches the
channels-first HBM layout; no input transpose). LayerNorm is folded into the
projections: q = rs_i * (x @ Wq' - mu_i * colsum(Wq')), with Wq' pre-scaled on
the host; k/v analogous (rank-1 -colsum*mu matmul accumulated into the same
psum group). Attention runs
in sim^T layout (keys on partitions, query tokens on the free axis): kT is
pre-scaled by rs_j so softmax is a plain exp; the denominator comes free as a
ones-column appended to v (row 64 of the out psum).

The softmax exp is SPLIT across two engines per j-tile BY HEAD: ACT runs
exact Exp on one head's 512 columns, DVE runs a Schraudolph bf16 exp
(i16 = round(s*A + B) bitcast to bf16, ~3% rel err) on the other head's,
into two separate tiles so the writes carry no ordering edge and each attn.v
matmul waits on exactly one engine. Sides rotate with hg parity so every
token sees Schraudolph on half its heads. The 128-token tail chunk runs as
ONE packed pass (4 head-groups side by side, 512-wide sims/avs).
Normalization: rec = exp(-ln(den)) on ACT straight from the PSUM denominator
row, bf16 PE broadcast, one DVE multiply. x arrives bf16 from the host (the
residual picks up a 0.4% x-rounding, within budget); k and v project in one
[128,n] psum (k rows 0:64, v rows 64:128). The y chain and the LN2 squares
run on the otherwise-idle Pool engine from SBUF-evacuated operands. A 14-
matmul warm-up burst gated on the first x slice flips the PE HAM clock gate
to 2.4 GHz before the projection pipeline starts. The preamble pipelines six
384-token x DMA slices against LN1 stats and the kv projection; deferred
tail phases (out-proj, LN2, y) run one chunk late, each popped BEFORE the
next head-group so they never trail the psum ring.

Precision: bf16 matmul operands with fp32 PSUM accumulation everywhere
(including stat/broadcast matmuls); LN statistics and the residual path stay
fp32. Softmax weights carry the Schraudolph error on DVE tiles, which keeps
the end-to-end absmax rel err ~1e-2 (gate: 2e-2).

Walrus quirks handled: one sync-wait per TPB instruction
(_split_multi_waits); --enable-ldw-opt=true and DVE ALU divide both crash
walrus codegen, so neither is used.
"""

import sys

import numpy as np

if "/opt/trn_rl_repo" not in sys.path:
    sys.path.insert(0, "/opt/trn_rl_repo")

C = 512          # channels
N = 2304         # tokens per batch (48*48)
NH = 1152        # tokens per core
HEADS = 8
DH = 64
CTXL = 77
CTXD = 768
JT = 19          # j tiles of 128: 18 img + 1 (ctx 0:77 | null 77 | pad)
JP = JT * 128
CHUNKS = [(0, 512), (512, 512), (1024, 128)]  # (start, len) token chunks
NCH = len(CHUNKS)
KT = 4           # C / 128
EPS = 1e-5

PROFILE = False
PROFILE_DIR = None

_cached = {}


def _split_multi_waits(nc):
    """Walrus codegen supports one sync-wait per TPB instruction (the EVENTS
    struct has a single wait slot). Tile attaches several. Split the extras
    onto same-engine NoOps inserted just before each instruction."""
    import concourse.mybir as mybir

    n = 0
    for fn in nc.m.functions:
        for bb in fn.blocks:
            insts = bb.instructions
            i = 0
            while i < len(insts):
                ins = insts[i]
                si = getattr(ins, "sync_info", None)
                if si is not None and si.on_wait and len(si.on_wait) > 1:
                    waits = list(si.on_wait)
                    for w in waits[:-1]:
                        n += 1
                        nop = mybir.InstNoOp(name=f"WSPLIT-{n}", engine=ins.engine)
                        nop.sync_info = mybir.SyncInfo(on_wait=[w], on_update=[])
                        insts.insert(i, nop)
                        i += 1
                    ins.sync_info = mybir.SyncInfo(
                        on_wait=[waits[-1]], on_update=si.on_update)
                i += 1
    return n


def _build_bass():
    import concourse.bass as bass
    import concourse.mybir as mybir
    import concourse.tile as tile
    from concourse.masks import make_identity
    from contextlib import ExitStack

    F32 = mybir.dt.float32
    BF = mybir.dt.bfloat16
    I16 = mybir.dt.int16
    AF = mybir.ActivationFunctionType
    ALU = mybir.AluOpType
    # Schraudolph bf16 exp: i16 = round(s*A + B), bitcast to bf16.
    SCHRA_A = float(2 ** 7 / np.log(2.0))
    SCHRA_B = float(127 * 2 ** 7 - 5.0)
    # Softmax exp is split WITHIN each j-tile by head: one head's 512 token
    # columns get exact Exp on ACT, the other head's get Schraudolph on DVE,
    # sides rotating with hg parity so every token sees Schraudolph on
    # exactly half of its heads. The pure per-head split keeps each attn.v
    # matmul dependent on a SINGLE exp engine (a mid-head split chains every
    # av onto ACT completion and serializes the loop).

    nc = bass.Bass()
    x_own = nc.declare_dram_parameter("x_own", [C, NH], BF, isOutput=False)
    x_oth = nc.declare_dram_parameter("x_oth", [C, NH], BF, isOutput=False)
    ctxt = nc.declare_dram_parameter("ctxt", [CTXL, CTXD], F32, isOutput=False)
    wq = nc.declare_dram_parameter("wq", [C, C], BF, isOutput=False)
    negcq = nc.declare_dram_parameter("negcq", [1, C], BF, isOutput=False)
    wkv = nc.declare_dram_parameter("wkv", [C, 2 * DH], BF, isOutput=False)
    ncskv = nc.declare_dram_parameter("ncskv", [1, 2 * DH], BF, isOutput=False)
    wctx = nc.declare_dram_parameter("wctx", [CTXD, 2 * DH], F32, isOutput=False)
    bctxk = nc.declare_dram_parameter("bctxk", [DH, 1], F32, isOutput=False)
    bctxv = nc.declare_dram_parameter("bctxv", [DH, 1], F32, isOutput=False)
    nullkt = nc.declare_dram_parameter("nullkt", [DH, 1], F32, isOutput=False)
    nullv = nc.declare_dram_parameter("nullv", [DH, 1], F32, isOutput=False)
    wout = nc.declare_dram_parameter("wout", [DH, HEADS * C], BF, isOutput=False)
    outg = nc.declare_dram_parameter("outg", [128, KT], F32, isOutput=False)
    y = nc.declare_dram_parameter("y", [C, NH], F32, isOutput=True)

    with tile.TileContext(nc) as tc, ExitStack() as ctx:
        pconst = ctx.enter_context(tc.tile_pool(name="const", bufs=1))
        pbig = ctx.enter_context(tc.tile_pool(name="big", bufs=1))

        ident = pconst.tile([128, 128], F32)
        make_identity(nc, ident[:])
        ident_bf = pconst.tile([128, 128], BF)
        make_identity(nc, ident_bf[:])
        # 64x64 identity living on partitions 64:128 (moving operand for the
        # v transpose, whose stationary input sits on the upper partitions)
        ident_hi = pconst.tile([128, 64], BF)
        make_identity(nc, ident_hi[64:128, 0:64])
        ones_col = pconst.tile([128, 1], F32)
        nc.vector.memset(ones_col[:], 1.0)
        ones_blk_bf = pconst.tile([128, 128], BF)
        nc.vector.memset(ones_blk_bf[:], 1.0)
        eps_col = pconst.tile([128, 1], F32)
        nc.vector.memset(eps_col[:], EPS)

        x_bf = pbig.tile([128, KT * N], BF)          # kt-major; own rows first
        # outg broadcast to full planes once, so the Pool-engine y chain
        # needs only tensor_tensor ops (Pool tensor_scalar is ~6x slower)
        outg_pl = pbig.tile([128, KT * 512], F32)
        # head-pair blocks + a packed block for the 128-token tail chunk
        # (all 4 hgs' last-128 tokens side by side -> full-width matmuls)
        QP = (HEADS // 2) * NH
        qT = pbig.tile([128, QP + 512], BF)
        kT2 = pbig.tile([128, JP], BF)               # rs-scaled keys, both halves
        v_sb = pbig.tile([128, JT * (DH + 1)], BF)   # per j-tile [128, 64+ones]
        projT = pbig.tile([128, KT * NH], BF)
        stats = pbig.tile([128, 40], F32)            # col jt: rs_j (v scaling)
        wout_sb = pbig.tile([64, HEADS * C], BF)
        outg_sb = pbig.tile([128, KT], F32)
        # per-token stat rows on partition 0: mu 0:N | rs N:2N
        # (LN2 reuses per cc: mu2 at cc*CH, rs2 at N+cc*CH, ex2 at 2N+cc*CH)
        rows = pbig.tile([1, 2 * N + NH], F32)
        rows_bf = pbig.tile([1, N], BF)
        R_RS, R_SC = N, 2 * N



        with tc.tile_pool(name="load", bufs=1) as pload, \
             tc.tile_pool(name="x2p", bufs=2) as px2, \
             tc.tile_pool(name="pss", bufs=2, space="PSUM") as pss:
            # setup psum tags: b1 [<=64,384]x2, bS [128,<=512]x4, bT [128,128]x2
            wq_sb = pload.tile([128, KT * C], BF)
            wkv_sb = pload.tile([128, KT * 2 * DH], BF)
            wctx_sb = pload.tile([128, CTXD], F32)
            negcq_sb = pload.tile([1, C], BF)
            ncskv_sb = pload.tile([1, 2 * DH], BF)
            bctxk_sb = pload.tile([DH, 1], F32)
            bctxv_sb = pload.tile([DH, 1], F32)
            vT = pload.tile([128, N], BF)     # v rows live at partitions 64:128
            ck_sb = pload.tile([64, CTXL], F32)
            cv_sb = pload.tile([64, CTXL + 1], F32)
            nullk_st = pload.tile([DH, 1], F32)
            nullv_st = pload.tile([DH, 1], F32)
            ctx_sb = pload.tile([CTXL, CTXD], F32)
            ctxnT = pload.tile([128, 6 * CTXL], F32)
            ex2 = pload.tile([1, N], F32)

            x_v = x_bf[:].rearrange("p (k n) -> p k n", k=KT)
            # Three DGE queues (SP/Pool/ACT). Queue heads carry what unblocks
            # compute first: x slice 0 (stats pipeline), ctx (LN chain), wctx
            # (ctx projection). Weights follow behind the first x slices.
            xo = [x_own[:, a : a + 384].rearrange("(k p) n -> p k n", p=128)
                  for a in (0, 384, 768)]
            xt = [x_oth[:, a : a + 384].rearrange("(k p) n -> p k n", p=128)
                  for a in (0, 384, 768)]
            nc.sync.dma_start(x_v[:, :, 0:384], xo[0])
            nc.gpsimd.dma_start(ctx_sb[:], ctxt[:, :])
            nc.gpsimd.dma_start(x_v[:, :, 384:768], xo[1])
            nc.scalar.dma_start(wctx_sb[:].rearrange("p (k n) -> p k n", k=6),
                                wctx[:].rearrange("(k p) n -> p k n", p=128))
            nc.scalar.dma_start(x_v[:, :, 768:1152], xo[2])
            nc.sync.dma_start(wkv_sb[:].rearrange("p (k n) -> p k n", k=KT),
                              wkv[:].rearrange("(k p) n -> p k n", p=128))
            nc.sync.dma_start(ncskv_sb[:], ncskv[:, :])
            nc.sync.dma_start(wq_sb[:].rearrange("p (k n) -> p k n", k=KT),
                              wq[:].rearrange("(k p) n -> p k n", p=128))
            nc.sync.dma_start(negcq_sb[:], negcq[:, :])
            nc.gpsimd.dma_start(nullk_st[:], nullkt[:, :])
            nc.gpsimd.dma_start(nullv_st[:], nullv[:, :])
            nc.scalar.dma_start(bctxk_sb[:], bctxk[:, :])
            nc.scalar.dma_start(bctxv_sb[:], bctxv[:, :])
            nc.sync.dma_start(x_v[:, :, NH : NH + 384], xt[0])
            nc.gpsimd.dma_start(x_v[:, :, NH + 384 : NH + 768], xt[1])
            nc.scalar.dma_start(x_v[:, :, NH + 768 : NH + 1152], xt[2])
            nc.scalar.dma_start(wout_sb[:], wout[:, :])
            nc.scalar.dma_start(outg_sb[:], outg[:, :])

            # ---- context: LN (layout A, bn_stats) + k/v projection ----
            cstat = pload.tile([CTXL, 3, 6], F32)
            for sg in range(3):
                nc.vector.bn_stats(cstat[:, sg, :],
                                   ctx_sb[:, sg * 256 : (sg + 1) * 256])
            cmv = pload.tile([CTXL, 2], F32)
            nc.vector.bn_aggr(cmv[:], cstat[:])
            nc.scalar.activation(cmv[:, 1:2], cmv[:, 1:2], AF.Ln,
                                 bias=eps_col[0:CTXL, :])
            nc.scalar.activation(cmv[:, 1:2], cmv[:, 1:2], AF.Exp, scale=-0.5)
            nc.vector.tensor_scalar(
                out=ctx_sb[:], in0=ctx_sb[:],
                scalar1=cmv[:, 0:1], scalar2=cmv[:, 1:2],
                op0=ALU.subtract, op1=ALU.mult)
            for kt in range(6):
                ps_ct = pss.tile([128, 128], F32, tag="bT")
                nc.tensor.transpose(ps_ct[:, 0:CTXL],
                                    ctx_sb[:, kt * 128 : (kt + 1) * 128],
                                    ident[:CTXL, :CTXL])
                nc.vector.tensor_copy(ctxnT[:, kt * CTXL : (kt + 1) * CTXL],
                                      ps_ct[:, 0:CTXL])
            ps_ck = pss.tile([64, 384], F32, tag="b1")
            ps_cv = pss.tile([64, 384], F32, tag="b1")
            for kt in range(6):
                nc.tensor.matmul(ps_ck[:, 0:CTXL],
                                 wctx_sb[:, kt * 128 : kt * 128 + DH],
                                 ctxnT[:, kt * CTXL : (kt + 1) * CTXL],
                                 start=(kt == 0), stop=(kt == 5))
                nc.tensor.matmul(ps_cv[:, 0:CTXL],
                                 wctx_sb[:, kt * 128 + DH : (kt + 1) * 128],
                                 ctxnT[:, kt * CTXL : (kt + 1) * CTXL],
                                 start=(kt == 0), stop=(kt == 5))
            # ---- PE clock warm-up: HAM throttles the PE to 1.2 GHz until it
            # sees ~3.4us of sustained activity. Burn fp32 matmuls on the
            # first x slice (dep => they start right as x lands, ~14us) so
            # the real projection pipeline runs at 2.4 GHz from the start.
            for _ in range(14):
                ps_w = pss.tile([1, 384], F32, tag="warm")
                nc.tensor.matmul(ps_w[0:1, :], ones_blk_bf[:, 0:1],
                                 x_bf[:, 0:384], start=True, stop=True)

            nc.vector.tensor_scalar_add(ck_sb[:], ps_ck[:, 0:CTXL], bctxk_sb[:])
            nc.vector.tensor_scalar_add(cv_sb[:, 0:CTXL], ps_cv[:, 0:CTXL],
                                        bctxv_sb[:])
            nc.vector.tensor_copy(cv_sb[:, CTXL : CTXL + 1], nullv_st[:])

            # ---- j-tile 18: [ctx 0:77 | null 77 | pad 78:128] ----
            VB = 18 * (DH + 1)
            nc.vector.memset(kT2[0:64, 18 * 128 : JP], 0.0)
            nc.vector.tensor_copy(kT2[0:64, 18 * 128 : 18 * 128 + CTXL], ck_sb[:])
            nc.vector.tensor_copy(kT2[0:64, 18 * 128 + CTXL : 18 * 128 + CTXL + 1],
                                  nullk_st[:])
            nc.vector.memset(v_sb[:, VB : VB + DH + 1], 0.0)
            ps_cvt = pss.tile([128, 128], F32, tag="bT")
            nc.tensor.transpose(ps_cvt[0 : CTXL + 1, 0:64], cv_sb[:],
                                ident[:64, :64])
            nc.vector.tensor_copy(v_sb[0 : CTXL + 1, VB : VB + DH],
                                  ps_cvt[0 : CTXL + 1, 0:64])
            nc.vector.memset(v_sb[0 : CTXL + 1, VB + DH : VB + DH + 1], 1.0)
            nc.gpsimd.dma_start(kT2[64:128, 18 * 128 : JP],
                                kT2[0:64, 18 * 128 : JP])

            # ---- per-384-token pipeline: stats -> kv proj -> v tiles ----
            # (each stage consumes one x DMA slice as it lands)
            ones_col_bf = pconst.tile([128, 1], BF)
            nc.vector.memset(ones_col_bf[:], 1.0)
            rs_bf = pload.tile([1, N], BF)

            def qproj(a0, ln, packed=False):
                # q projection for one token chunk (LN + 1/sqrt(dh) folded);
                # interleaved into the per-384 pipeline as soon as its token
                # span has stats, so q DVE work doesn't queue behind all kv.
                # packed=True lays the 4 head-groups side by side at QP
                # (tail chunk: one 512-wide attention pass over all 8 heads)
                sl = slice(a0, a0 + ln)
                ps_rs = pss.tile([128, 512], F32, tag="bS")
                nc.tensor.matmul(ps_rs[:, 0:ln], ones_blk_bf[0:1, :],
                                 rs_bf[0:1, sl])
                rs_b = px2.tile([128, 512], F32, tag="rsb")
                nc.vector.tensor_copy(rs_b[:, 0:ln], ps_rs[:, 0:ln])
                for hg in range(HEADS // 2):
                    ps_q = pss.tile([128, 512], F32, tag="bS")
                    for kt in range(KT):
                        nc.tensor.matmul(
                            ps_q[:, 0:ln],
                            wq_sb[:, kt * C + hg * 128 : kt * C + (hg + 1) * 128],
                            x_bf[:, kt * N + a0 : kt * N + a0 + ln],
                            start=(kt == 0), stop=False)
                    nc.tensor.matmul(ps_q[:, 0:ln],
                                     negcq_sb[0:1, hg * 128 : (hg + 1) * 128],
                                     rows_bf[0:1, sl], start=False, stop=True)
                    d0 = QP + hg * 128 if packed else hg * NH + a0
                    nc.vector.tensor_mul(
                        qT[:, d0 : d0 + ln],
                        ps_q[:, 0:ln], rs_b[:, 0:ln])

            for ch in range(6):
                sl = slice(ch * 384, (ch + 1) * 384)
                ps_r1 = pss.tile([64, 384], F32, tag="b1")
                for kt in range(KT):
                    nc.tensor.matmul(
                        ps_r1[0:1, :], ones_col_bf[:],
                        x_bf[:, kt * N + ch * 384 : kt * N + (ch + 1) * 384],
                        start=(kt == 0), stop=(kt == KT - 1))
                nc.scalar.mul(rows[0:1, sl], ps_r1[0:1, :], 1.0 / C)
                nc.vector.tensor_copy(rows_bf[0:1, sl], rows[0:1, sl])
                x2 = px2.tile([128, KT * 384], BF, tag="x2")
                ps_r2 = pss.tile([64, 384], F32, tag="b1")
                for kt in range(KT):
                    xs = x_bf[:, kt * N + ch * 384 : kt * N + (ch + 1) * 384]
                    nc.gpsimd.tensor_mul(x2[:, kt * 384 : (kt + 1) * 384], xs, xs)
                    nc.tensor.matmul(
                        ps_r2[0:1, :], ones_col_bf[:],
                        x2[:, kt * 384 : (kt + 1) * 384],
                        start=(kt == 0), stop=(kt == KT - 1))
                nc.scalar.mul(ex2[0:1, ch * 384 : (ch + 1) * 384],
                              ps_r2[0:1, :], 1.0 / C)
                a, b = R_RS + ch * 384, R_RS + (ch + 1) * 384
                mu = rows[0:1, ch * 384 : (ch + 1) * 384]
                nc.vector.tensor_mul(rows[0:1, a:b], mu, mu)
                nc.vector.tensor_sub(rows[0:1, a:b],
                                     ex2[0:1, ch * 384 : (ch + 1) * 384],
                                     rows[0:1, a:b])
                nc.scalar.activation(rows[0:1, a:b], rows[0:1, a:b], AF.Ln,
                                     bias=eps_col[0:1, :])
                nc.scalar.activation(rows[0:1, a:b], rows[0:1, a:b], AF.Exp,
                                     scale=-0.5)
                nc.vector.tensor_copy(
                    rs_bf[0:1, sl],
                    rows[0:1, R_RS + ch * 384 : R_RS + (ch + 1) * 384])
                # rs as per-partition columns (v scaling), 3 j-tiles per ch
                for jt in range(ch * 3, ch * 3 + 3):
                    ps_c = pss.tile([128, 128], F32, tag="bT")
                    nc.tensor.matmul(
                        ps_c[:, 0:1],
                        rows[0:1, R_RS + jt * 128 : R_RS + (jt + 1) * 128],
                        ones_col[0:1, :])
                    nc.vector.tensor_copy(stats[:, jt : jt + 1], ps_c[:, 0:1])
                # kv projection for this 384-token slice (LN folded): k and
                # v land in one [128, n] psum (k rows 0:64, v rows 64:128)
                kva, kvl = ch * 384, 384
                ps_kv = pss.tile([128, 512], F32, tag="bS")
                for kt in range(KT):
                    xs = x_bf[:, kt * N + kva : kt * N + kva + kvl]
                    nc.tensor.matmul(ps_kv[0:128, 0:kvl],
                                     wkv_sb[:, kt * 128 : (kt + 1) * 128],
                                     xs, start=(kt == 0), stop=False)
                nc.tensor.matmul(ps_kv[0:128, 0:kvl], ncskv_sb[:],
                                 rows_bf[0:1, sl], start=False, stop=True)
                ps_bc = pss.tile([128, 512], F32, tag="bS")
                nc.tensor.matmul(ps_bc[0:64, 0:kvl], ones_blk_bf[0:1, 0:64],
                                 rs_bf[0:1, sl])
                kk = px2.tile([64, 512], F32, tag="kk")
                nc.vector.tensor_copy(kk[:, 0:kvl], ps_kv[0:64, 0:kvl])
                nc.vector.tensor_mul(kT2[0:64, sl], kk[:, 0:kvl],
                                     ps_bc[0:64, 0:kvl])
                # duplicate this slice to partitions 64:128 right away, so
                # attention isn't gated on the LAST kv slice + one big copy
                nc.sync.dma_start(kT2[64:128, sl], kT2[0:64, sl])
                nc.vector.tensor_copy(vT[64:128, sl], ps_kv[64:128, 0:kvl])
                # v tiles: transpose + rs scale + ones col
                for jt in range(ch * 3, ch * 3 + 3):
                    ps_vt = pss.tile([128, 128], BF, tag="bT")
                    nc.tensor.transpose(ps_vt[:, 0:64],
                                        vT[64:128, jt * 128 : (jt + 1) * 128],
                                        ident_hi[64:128, 0:64])
                    vb = jt * (DH + 1)
                    nc.vector.tensor_scalar_mul(v_sb[:, vb : vb + DH],
                                                ps_vt[:, 0:64],
                                                stats[:, jt : jt + 1])
                    nc.vector.memset(v_sb[:, vb + DH : vb + DH + 1], 1.0)
                if ch == 1:
                    qproj(0, 512)
                elif ch == 2:
                    qproj(512, 512)
                    qproj(1024, 128, packed=True)

            # outg broadcast planes for the Pool-engine y chain (placed after
            # the pipeline so the in-order DVE queue never stalls on the late
            # outg DMA)
            nc.vector.memset(outg_pl[:, 0:512], 1.0)
            for ct in range(1, KT):
                nc.vector.tensor_copy(outg_pl[:, ct * 512 : (ct + 1) * 512],
                                      outg_pl[:, 0:512])
            for ct in range(KT):
                nc.vector.tensor_scalar_mul(
                    outg_pl[:, ct * 512 : (ct + 1) * 512],
                    outg_pl[:, ct * 512 : (ct + 1) * 512],
                    outg_sb[:, ct : ct + 1])

        # ========= attention + output + LN2 + residual, per chunk =========
        # Per (chunk, head-pair): row-packed sims -> exp (split ACT/DVE by
        # j-tile: ACT exact, DVE Schraudolph-bf16) -> attn.v pair lagging one
        # j-tile. Softmax normalization: rec = exp(-ln(den)) on ACT straight
        # from PSUM, bf16 PE broadcast, one DVE mul -> ot. The output tail
        # (out-proj, LN2, y) is DEFERRED one chunk so tail matmuls never
        # head-of-line-block the in-order PE queue.
        with tc.tile_pool(name="attn", bufs=6) as pattn, \
             tc.tile_pool(name="outp", bufs=16) as pout, \
             tc.tile_pool(name="recp", bufs=6) as prec, \
             tc.tile_pool(name="rbsp", bufs=4) as prbs, \
             tc.tile_pool(name="p2p", bufs=2) as pp2, \
             tc.tile_pool(name="yp", bufs=4) as pyt, \
             tc.tile_pool(name="rwbp", bufs=2) as prwb, \
             tc.tile_pool(name="psatt", bufs=2, space="PSUM") as psA, \
             tc.tile_pool(name="pspo", bufs=2, space="PSUM") as psB, \
             tc.tile_pool(name="psmx", bufs=1, space="PSUM") as psC:

            def exp_split(ps0, ps1, act_first):
                # one head's sim tile gets exact Exp on ACT, the other
                # Schraudolph on DVE. Separate PSUM sim tiles AND separate
                # output tiles per engine: any shared tile (even disjoint
                # halves) gives the dep tracker an ACT->DVE edge that
                # serializes the two exps. Returns (mv0, mv1) moving APs
                # for the two attn.v matmuls.
                atA = pattn.tile([128, 512], BF, tag="atA")
                atB = pattn.tile([128, 512], I16, tag="atB")
                psa, psd = (ps0, ps1) if act_first else (ps1, ps0)
                nc.scalar.activation(atA[:], psa[:, 0:512], AF.Exp)
                nc.vector.tensor_scalar(
                    out=atB[:], in0=psd[:, 0:512],
                    scalar1=SCHRA_A, scalar2=SCHRA_B,
                    op0=ALU.mult, op1=ALU.add)
                if act_first:
                    return atA[:], atB[:].bitcast(BF)
                return atB[:].bitcast(BF), atA[:]

            def normalize_act(po, ln):
                # rec = exp(-ln(den)) on ACT from the PSUM denominator row
                recb = prec.tile([65, 512], F32, tag="rec")
                nc.scalar.activation(recb[64:65, 0:ln], po[64:65, 0:ln],
                                     AF.Ln, bias=eps_col[64:65, :])
                rec_bf = prec.tile([65, 512], BF, tag="recb")
                nc.scalar.activation(rec_bf[64:65, 0:ln],
                                     recb[64:65, 0:ln], AF.Exp, scale=-1.0)
                return rec_bf

            def normalize_pe(po, rec_bf, ln, ots):
                # bf16 PE broadcast of rec, one DVE mul -> ot. Deferred one
                # j-tile into the NEXT head-group so the broadcast never
                # heads the PE queue while ACT still computes rec (that
                # stall idled the PE every boundary and tripped the HAM
                # clock gate down to 1.2 GHz).
                ps_rb = psC.tile([128, 512], F32, tag="mx")
                nc.tensor.matmul(ps_rb[0:64, 0:ln],
                                 ones_blk_bf[64:65, 0:64],
                                 rec_bf[64:65, 0:ln],
                                 start=True, stop=True)
                rb_sb = prbs.tile([64, 512], F32, tag="rbs")
                nc.vector.tensor_copy(rb_sb[:, 0:ln], ps_rb[0:64, 0:ln])
                ot = pout.tile([64, 512], BF, tag="ot")
                nc.vector.tensor_mul(ot[:, 0:ln], po[0:64, 0:ln],
                                     rb_sb[:, 0:ln])
                ots.append(ot)

            def normalize(po, ln, ots):
                normalize_pe(po, normalize_act(po, ln), ln, ots)

            def run_hg(q0, q1, ln, ots, hg, pending=None):
                po0 = psB.tile([65, 512], F32, tag="po")
                po1 = psB.tile([65, 512], F32, tag="po")
                po = [po0, po1]
                off1 = 512
                ats = [None] * JT
                for jt in range(JT):
                    if jt == 1 and pending is not None:
                        pending()
                        pending = None
                    ps0 = psA.tile([128, 512], F32, tag="simA")
                    ps1 = psA.tile([128, 512], F32, tag="simB")
                    nc.tensor.matmul(ps0[:, 0:ln],
                                     kT2[0:64, jt * 128 : (jt + 1) * 128],
                                     q0, start=True, stop=True)
                    nc.tensor.matmul(ps1[:, 0:ln],
                                     kT2[64:128, jt * 128 : (jt + 1) * 128],
                                     q1, start=True, stop=True)
                    # ln == 512 here (tail chunk runs packed in run_tail)
                    ats[jt] = exp_split(ps0, ps1, act_first=(hg % 2 == 0))
                    if jt > 0:
                        j0 = jt - 1
                        vs = v_sb[:, j0 * (DH + 1) : (j0 + 1) * (DH + 1)]
                        nc.tensor.matmul(po[0][0:65, 0:ln], vs,
                                         ats[j0][0][:, 0:ln],
                                         start=(j0 == 0), stop=False)
                        nc.tensor.matmul(po[1][0:65, 0:ln], vs,
                                         ats[j0][1][:, 0:ln],
                                         start=(j0 == 0), stop=False)
                        ats[j0] = None
                j0 = JT - 1
                vs = v_sb[:, j0 * (DH + 1) : (j0 + 1) * (DH + 1)]
                nc.tensor.matmul(po[0][0:65, 0:ln], vs,
                                 ats[j0][0][:, 0:ln],
                                 start=False, stop=True)
                nc.tensor.matmul(po[1][0:65, 0:ln], vs,
                                 ats[j0][1][:, 0:ln],
                                 start=False, stop=True)
                # ACT part now (runs behind the exp backlog), PE/DVE part
                # deferred into the next head-group's loop
                rbf0 = normalize_act(po[0], ln)
                rbf1 = normalize_act(po[1], ln)

                def finish():
                    normalize_pe(po[0], rbf0, ln, ots)
                    normalize_pe(po[1], rbf1, ln, ots)

                return finish

            def run_tail(phases, pending=None):
                # tail chunk (tokens 1024:1152): ONE pass over the j-tiles
                # with all 4 head-groups packed side by side (128 cols each):
                # cols 0:512 first-of-pair heads, 512:1024 second-of-pair.
                ots = []
                poA = psB.tile([65, 512], F32, tag="po")
                poB = psB.tile([65, 512], F32, tag="po")
                ats = [None] * JT
                for jt in range(JT):
                    if jt == 1 and pending is not None:
                        pending()
                        pending = None
                    # pop only mx-backed phases (proj/ln2) inside the pass;
                    # the y phase touches the po ring poA/poB live on
                    if len(phases) > 1 and jt in (2, 5, 8, 11, 14):
                        phases.pop(0)()
                    ps0 = psA.tile([128, 512], F32, tag="simA")
                    ps1 = psA.tile([128, 512], F32, tag="simB")
                    nc.tensor.matmul(ps0[:, 0:512],
                                     kT2[0:64, jt * 128 : (jt + 1) * 128],
                                     qT[0:64, QP : QP + 512],
                                     start=True, stop=True)
                    nc.tensor.matmul(ps1[:, 0:512],
                                     kT2[64:128, jt * 128 : (jt + 1) * 128],
                                     qT[64:128, QP : QP + 512],
                                     start=True, stop=True)
                    ats[jt] = exp_split(ps0, ps1, act_first=True)
                    if jt > 0:
                        j0 = jt - 1
                        vs = v_sb[:, j0 * (DH + 1) : (j0 + 1) * (DH + 1)]
                        nc.tensor.matmul(poA[0:65, 0:512], vs,
                                         ats[j0][0][:, 0:512],
                                         start=(j0 == 0), stop=False)
                        nc.tensor.matmul(poB[0:65, 0:512], vs,
                                         ats[j0][1][:, 0:512],
                                         start=(j0 == 0), stop=False)
                        ats[j0] = None
                j0 = JT - 1
                vs = v_sb[:, j0 * (DH + 1) : (j0 + 1) * (DH + 1)]
                nc.tensor.matmul(poA[0:65, 0:512], vs,
                                 ats[j0][0][:, 0:512],
                                 start=False, stop=True)
                nc.tensor.matmul(poB[0:65, 0:512], vs,
                                 ats[j0][1][:, 0:512],
                                 start=False, stop=True)
                normalize(poA, 512, ots)
                normalize(poB, 512, ots)
                return ots

            def tail_phases(a0, ln, ots, packed=False, last=False):
                def ph_proj(ct):
                    ps_p = psC.tile([128, 512], F32, tag="mx")
                    for h in range(HEADS):
                        if packed:
                            mv = ots[h % 2][:, (h // 2) * 128 :
                                            (h // 2) * 128 + 128]
                        else:
                            mv = ots[h][:, 0:ln]
                        nc.tensor.matmul(
                            ps_p[:, 0:ln],
                            wout_sb[:, h * C + ct * 128 : h * C + (ct + 1) * 128],
                            mv,
                            start=(h == 0), stop=(h == HEADS - 1))
                    nc.vector.tensor_copy(
                        projT[:, ct * NH + a0 : ct * NH + a0 + ln],
                        ps_p[:, 0:ln])

                def ph_ln2():
                    ra, rb2 = R_RS + a0, R_RS + a0 + ln
                    sca, scb = R_SC + a0, R_SC + a0 + ln
                    ps_m2 = psC.tile([128, 512], F32, tag="mx")
                    for ct in range(KT):
                        nc.tensor.matmul(
                            ps_m2[0:1, 0:ln], ones_col_bf[:],
                            projT[:, ct * NH + a0 : ct * NH + a0 + ln],
                            start=(ct == 0), stop=(ct == KT - 1))
                    nc.scalar.mul(rows[0:1, a0 : a0 + ln], ps_m2[0:1, 0:ln], 1.0 / C)
                    p2 = pp2.tile([128, KT * 512], BF, tag="p2")
                    ps_q2 = psC.tile([128, 512], F32, tag="mx")
                    sq_eng = nc.vector if last else nc.gpsimd
                    for ct in range(KT):
                        pslc = projT[:, ct * NH + a0 : ct * NH + a0 + ln]
                        sq_eng.tensor_mul(p2[:, ct * 512 : ct * 512 + ln],
                                          pslc, pslc)
                        nc.tensor.matmul(ps_q2[0:1, 0:ln], ones_col_bf[:],
                                         p2[:, ct * 512 : ct * 512 + ln],
                                         start=(ct == 0), stop=(ct == KT - 1))
                    nc.scalar.mul(rows[0:1, sca:scb], ps_q2[0:1, 0:ln], 1.0 / C)
                    nc.vector.tensor_mul(rows[0:1, ra:rb2], rows[0:1, a0 : a0 + ln],
                                         rows[0:1, a0 : a0 + ln])
                    nc.vector.tensor_sub(rows[0:1, ra:rb2], rows[0:1, sca:scb],
                                         rows[0:1, ra:rb2])
                    nc.scalar.activation(rows[0:1, ra:rb2], rows[0:1, ra:rb2],
                                         AF.Ln, bias=eps_col[0:1, :])
                    nc.scalar.activation(rows[0:1, ra:rb2], rows[0:1, ra:rb2],
                                         AF.Exp, scale=-0.5)
                    rwb = prwb.tile([1, 1024], BF, tag="rwb")
                    nc.vector.tensor_copy(rwb[0:1, 0:ln], rows[0:1, a0 : a0 + ln])
                    nc.vector.tensor_copy(rwb[0:1, 512 : 512 + ln],
                                          rows[0:1, ra:rb2])
                    return rwb

                def ph_y(rwb):
                    # bm/br must NOT take po-ring slots: a pop could evict a
                    # po whose deferred normalize_pe hasn't been emitted yet
                    # (its broadcast would then sit behind us in the PE
                    # queue -> deadlock)
                    ps_bm = psB.tile([128, 512], F32, tag="ybc", bufs=1)
                    nc.tensor.matmul(ps_bm[:, 0:ln], ones_blk_bf[0:1, :],
                                     rwb[0:1, 0:ln], start=True, stop=True)
                    ps_br = psC.tile([128, 512], F32, tag="mx")
                    nc.tensor.matmul(ps_br[:, 0:ln], ones_blk_bf[0:1, :],
                                     rwb[0:1, 512 : 512 + ln],
                                     start=True, stop=True)
                    # evacuate broadcasts to SBUF so the y chain can run on
                    # the (otherwise idle) Pool engine, which can't read PSUM
                    bmr = pyt.tile([128, 1024], F32, tag="bmr")
                    nc.vector.tensor_copy(bmr[:, 0:ln], ps_bm[:, 0:ln])
                    nc.vector.tensor_copy(bmr[:, 512 : 512 + ln],
                                          ps_br[:, 0:ln])
                    for ct in range(KT):
                        # during attention the y chain runs on the idle Pool
                        # engine; for the final chunk (nothing left to
                        # overlap) split it across DVE and Pool for latency
                        eng = (nc.vector if ct % 2 == 0 else
                               nc.gpsimd) if last else nc.gpsimd
                        yt = pyt.tile([128, 512], F32, tag="yt")
                        pslice = projT[:, ct * NH + a0 : ct * NH + a0 + ln]
                        eng.tensor_sub(yt[:, 0:ln], pslice, bmr[:, 0:ln])
                        eng.tensor_mul(yt[:, 0:ln], yt[:, 0:ln],
                                       bmr[:, 512 : 512 + ln])
                        eng.tensor_mul(yt[:, 0:ln], yt[:, 0:ln],
                                       outg_pl[:, ct * 512 : ct * 512 + ln])
                        eng.tensor_add(
                            yt[:, 0:ln], yt[:, 0:ln],
                            x_bf[:, ct * N + a0 : ct * N + a0 + ln])
                        nc.sync.dma_start(
                            y[ct * 128 : (ct + 1) * 128, a0 : a0 + ln],
                            yt[:, 0:ln])

                state = {}

                def s_ln2():
                    state["rwb"] = ph_ln2()

                def s_y():
                    ph_y(state["rwb"])

                # six phases: 4 per-ct out-projections (PSUM tag mx only —
                # safe to pop anywhere), ln2 (mx), y (takes po-ring slots:
                # only safe to pop at head-group boundaries, never mid-loop)
                return [lambda ct=ct: ph_proj(ct) for ct in range(KT)] + \
                    [s_ln2, s_y]

            phases = []
            pending = None
            for a0, ln in CHUNKS[:2]:
                ots = []
                for hg in range(HEADS // 2):
                    # pop TWO phases at boundary slots 1..3 (slot 0 would
                    # make ph_proj wait on the previous chunk's final
                    # head-group normalization); mid-loop injection stalls
                    # the attention pipeline, so boundaries only
                    if phases and hg > 0:
                        phases.pop(0)()
                        if phases:
                            phases.pop(0)()
                    pending = run_hg(
                        qT[0:64, hg * NH + a0 : hg * NH + a0 + ln],
                        qT[64:128, hg * NH + a0 : hg * NH + a0 + ln],
                        ln, ots, hg, pending=pending)
                phases = tail_phases(a0, ln, ots)
            ots = run_tail(phases, pending=pending)
            # remaining phase of chunk 1 (y) overlaps the tail's own phases
            for ph in phases:
                ph()
            phases = tail_phases(1024, 128, ots, packed=True, last=True)
            for ph in phases:
                ph()
    _split_multi_waits(nc)
    return nc


def _prep_inputs(x, context, norm_gamma, null_kv, Wq, Wkv, ctx_ln_g, ctx_ln_b,
                 Wctx, bctx, Wout, out_ln_g):
    import ml_dtypes
    bf = ml_dtypes.bfloat16
    f = np.float32
    x = np.asarray(x, f).reshape(4, C, N)
    context = np.asarray(context, f)
    g = np.asarray(norm_gamma, f)
    scale = 1.0 / np.sqrt(DH)
    wq_h = (g[:, None] * np.asarray(Wq, f)) * scale
    negcq_h = -wq_h.sum(0, dtype=np.float64).astype(f)[None, :]
    wkv_h = g[:, None] * np.asarray(Wkv, f)
    ncsk_h = -wkv_h[:, :DH].sum(0, dtype=np.float64).astype(f)[None, :]
    ncsv_h = -wkv_h[:, DH:].sum(0, dtype=np.float64).astype(f)[None, :]
    wctx_h = np.asarray(ctx_ln_g, f)[:, None] * np.asarray(Wctx, f)
    bctx_h = (np.asarray(bctx, f) + np.asarray(ctx_ln_b, f) @ np.asarray(Wctx, f))
    null = np.asarray(null_kv, f)
    wout_b = np.concatenate(
        [np.asarray(Wout, f)[h * DH:(h + 1) * DH, :] for h in range(HEADS)], axis=1)
    outg_h = np.ascontiguousarray(np.asarray(out_ln_g, f).reshape(KT, 128).T)

    shared = {
        "wq": np.ascontiguousarray(wq_h).astype(bf),
        "negcq": negcq_h.astype(bf),
        "wkv": np.ascontiguousarray(wkv_h).astype(bf),
        "ncskv": np.concatenate([ncsk_h, ncsv_h], axis=1).astype(bf),
        "wctx": np.ascontiguousarray(wctx_h),
        "bctxk": np.ascontiguousarray(bctx_h[:DH, None]),
        "bctxv": np.ascontiguousarray(bctx_h[DH:, None]),
        "nullkt": np.ascontiguousarray(null[0][:, None]),
        "nullv": np.ascontiguousarray(null[1][:, None]),
        "wout": np.ascontiguousarray(wout_b).astype(bf),
        "outg": outg_h,
    }
    in_maps = []
    for core in range(8):
        b, half = core // 2, core % 2
        m = dict(shared)
        m["x_own"] = np.ascontiguousarray(
            x[b][:, half * NH : (half + 1) * NH]).astype(bf)
        m["x_oth"] = np.ascontiguousarray(
            x[b][:, (1 - half) * NH : (2 - half) * NH]).astype(bf)
        m["ctxt"] = np.ascontiguousarray(context[b])
        in_maps.append(m)
    return in_maps


def _register_profile_hook():
    if "antenv.axon_hooks" in sys.modules:
        return
    import types

    m = types.ModuleType("antenv.axon_hooks")
    m._hook = None

    def _set_hook(h, _m=m):
        _m._hook = h

    def _get_hook(_m=m):
        return _m._hook

    m.set_axon_ntff_profile_hook = _set_hook
    m.get_axon_ntff_profile_hook = _get_hook
    sys.modules["antenv.axon_hooks"] = m
    import antenv

    antenv.axon_hooks = m
    from trn_agent_boot.trn_boot import _ntff_profile_via_ctypes

    _set_hook(_ntff_profile_via_ctypes("/opt/axon/libaxon_pjrt.so"))


_LDW_OPT = [False]


def _patch_ldw_opt():
    import concourse.bass_utils as bu
    if getattr(bu, "_ldwopt_patched", False):
        return
    orig = bu.run_command

    def run2(cmd, **kw):
        if _LDW_OPT[0]:
            cmd = [c.replace("--enable-ldw-opt=false", "--enable-ldw-opt=true")
                   for c in cmd]
        return orig(cmd, **kw)

    bu.run_command = run2
    bu._ldwopt_patched = True


def kernel(**inputs):
    from concourse.bass_utils import run_bass_kernel_spmd
    _patch_ldw_opt()

    if "nc" not in _cached:
        _cached["nc"] = _build_bass()
    nc = _cached["nc"]
    in_maps = _prep_inputs(**inputs)
    kw = {}
    if PROFILE:
        _register_profile_hook()
        kw = dict(trace=True, tmpdir=PROFILE_DIR)
    res = run_bass_kernel_spmd(nc, in_maps, list(range(8)), **kw)
    _cached["last"] = res
    out = np.empty((4, C, N), np.float32)
    for core in range(8):
        b, half = core // 2, core % 2
        out[b][:, half * NH : (half + 1) * NH] = res.results[core]["y"]
    return out.reshape(4, C, 48, 48)

